# revision 9
# baseline (speedup 1.0000x reference)
"""GQA causal attention block (RoPE, 32 Q heads / 8 KV heads, S=2048, D=4096)
distributed tensor-parallel over heads across 8 TRN2 NeuronCores.

Per core c:
  - 4 query heads (wq cols 512c:512c+512), 1 KV head (wk/wv cols 128c:128c+128)
  - projections computed transposed (qT/kT/vT [hd, seq]) from host-transposed
    xT, weights as stationary operands, bf16 matmuls with f32 PSUM accum
  - RoPE applied with host tables (bf16); the half-rotation uses a PE
    permutation matmul (cross-partition moves are impossible on DVE)
  - attention group g (i-chunk g of 512) runs immediately after projection
    chunk g — causality means it only needs K/V through chunk g — so each
    chunked AllGather launches hundreds of us earlier and inter-core launch
    skew is absorbed by the remaining compute instead of stalling the PE
  - attention computed as S^T [j, i] tiles -> exp (ACT, scale folded in) ->
    P^T tiles feed the PV matmul directly; causal handled by skipping future
    j-tiles, streaming only the live column subrange on diagonal tiles, and
    one DVE 0/1 triangular mask multiply on the first 128 live columns; the
    l/oT accumulation trails st/exp by 3 tiles so the PE never blocks on the
    ACT exp latency
  - row sums via a ones-vector matmul; 1/l = exp(-ln l) computed entirely on
    ACT (keeps the in-order ACT queue free of cross-engine waits), broadcast
    across partitions with a K=1 PE matmul (no DRAM round-trip); the
    broadcast matmul is software-pipelined one head late
  - output projection computed transposed (outT = wo_c^T @ Y^T); outT is
    written bf16 and upcast on host
  - bulk DMA alternates between the SP and ACT issue queues, ordered so
    chunk-0 operands land first; the GpSimd queue carries ONLY the four
    AllGather triggers (a trigger parked on the serial CC stream under launch
    skew would otherwise head-of-line-block DMAs queued behind it)
Host gathers by concatenating the 8 (transposed) column shards.
"""

import numpy as np
import ml_dtypes

import concourse.bass as bass
import concourse.mybir as mybir
import concourse.tile as tile
from concourse.bass_utils import run_bass_kernel_spmd

bf16 = mybir.dt.bfloat16
f32 = mybir.dt.float32

NCORES = 8
S = 2048
DIM = 4096
HD = 128
NH = 32
QH = NH // NCORES          # 4 query heads per core
QW = QH * HD               # 512 wq cols per core
ROPE_BASE = 10000.0
SCALE = float(HD) ** -0.5
NSEQ = S // 512            # 4 seq chunks
KT = DIM // 128            # 32 contraction tiles

_CACHE = {}


def _legalize_waits(nc, allowed_default=1):
    """This walrus build rejects instructions carrying more inline sync waits
    than the opcode template allows (0 for Drain, 1 elsewhere). Spill excess
    waits onto standalone EventSemaphore instructions inserted immediately
    before, on the same engine (engine order preserves semantics)."""
    for f in nc.m.functions:
        for bb in f.blocks:
            out = []
            for ins in bb.instructions:
                tname = type(ins).__name__
                si = getattr(ins, "sync_info", None)
                waits = list(si.on_wait) if (si is not None and si.on_wait) else []
                if tname == "InstEventSemaphore":
                    allowed = len(waits)
                elif tname == "InstDrain":
                    allowed = 0
                else:
                    allowed = allowed_default
                if len(waits) > allowed:
                    spill, keep = waits[allowed:], waits[:allowed]
                    for i, w in enumerate(spill):
                        ev = mybir.InstEventSemaphore(
                            name=f"{ins.name}_wfix{i}",
                            engine=ins.engine, ins=[], outs=[],
                        )
                        ev.sync_info = mybir.SyncInfo(on_wait=[w], on_update=[])
                        out.append(ev)
                    si.on_wait = keep
                out.append(ins)
            bb.instructions[:] = out


def _build_nc():
    nc = bass.Bass(num_devices=NCORES)

    xT = nc.declare_dram_parameter("xT", [DIM, S], bf16, isOutput=False)
    wq = nc.declare_dram_parameter("wq", [DIM, QW], bf16, isOutput=False)
    wk = nc.declare_dram_parameter("wk", [DIM, HD], bf16, isOutput=False)
    wv = nc.declare_dram_parameter("wv", [DIM, HD], bf16, isOutput=False)
    wo = nc.declare_dram_parameter("wo", [DIM, QW], bf16, isOutput=False)
    cosT = nc.declare_dram_parameter("cosT", [HD, S], bf16, isOutput=False)
    sinT = nc.declare_dram_parameter("sinT", [HD, S], bf16, isOutput=False)
    tri = nc.declare_dram_parameter("tri", [128, 128], bf16, isOutput=False)
    perm = nc.declare_dram_parameter("perm", [128, 128], bf16, isOutput=False)
    ident = nc.declare_dram_parameter("ident", [128, 128], bf16, isOutput=False)
    outT = nc.declare_dram_parameter("outT", [QW, S], bf16, isOutput=True)

    ag_ins = [nc.dram_tensor(f"ag_in{g}", [QW, 512], bf16) for g in range(4)]
    ag_outs = [
        nc.dram_tensor(f"ag_out{g}", [NCORES, QW, 512], bf16, addr_space="Shared")
        for g in range(4)
    ]

    with tile.TileContext(nc) as tc:
        with (
            tc.tile_pool(name="const", bufs=1) as constp,
            tc.tile_pool(name="acts", bufs=1) as acts,
            tc.tile_pool(name="xin", bufs=6) as xin,
            tc.tile_pool(name="rope", bufs=2) as rope,
            tc.tile_pool(name="pt", bufs=6) as ptp,
            tc.tile_pool(name="epi", bufs=3) as epi,
            tc.tile_pool(name="cproj", bufs=12) as cproj,
            tc.tile_pool(name="psum", bufs=1, space="PSUM") as psum,
        ):
            def pbank(b, shape=(128, 512), dtype=f32, name="ps"):
                return psum.tile(list(shape), dtype, tag=f"b{b}",
                                 name=f"{name}_b{b}", bufs=1)

            # --- PE warmup: ~12 dummy matmuls on a memset tile fill the
            # 9-13us NEFF-startup window (before the first DMAs land) with
            # back-to-back PE work, flipping the HAM clock gate to K=16/16
            # (2.4GHz) BEFORE the real chunk-0 projections start; without
            # this the first ~70 matmuls run at the cold 1.2GHz clock ---
            warm_sb = constp.tile([128, 512], bf16)
            nc.vector.memset(warm_sb[:], 0.015625)
            warm_ps = pbank(7, name="warm")
            for _ in range(12):
                nc.tensor.matmul(warm_ps[:], warm_sb[:, 0:128], warm_sb[:],
                                 start=True, stop=True)

            # --- constants / weights (SBUF homes) ---
            wq_sb = constp.tile([128, KT, QW], bf16)
            wk_sb = constp.tile([128, KT, HD], bf16)
            wv_sb = constp.tile([128, KT, HD], bf16)
            wo_sb = constp.tile([128, KT, QW], bf16)
            cos_sb = constp.tile([HD, S], bf16)
            sin_sb = constp.tile([HD, S], bf16)
            tri_sb = constp.tile([128, 128], bf16)
            perm_sb = constp.tile([128, 128], bf16)
            ident_sb = constp.tile([128, 128], bf16)
            ones_sb = constp.tile([128, 1], bf16)
            onesr_sb = constp.tile([1, 128], bf16)
            nc.vector.memset(ones_sb[:], 1.0)
            nc.vector.memset(onesr_sb[:], 1.0)

            wqr = wq.rearrange("(a p) m -> p a m", p=128)
            wkr = wk.rearrange("(a p) m -> p a m", p=128)
            wvr = wv.rearrange("(a p) m -> p a m", p=128)
            wor = wo.rearrange("(a p) m -> p a m", p=128)
            xr = xT.rearrange("(a p) m -> p a m", p=128)

            # activations that live through attention
            qTr = acts.tile([128, QH, S], bf16)      # 4 head tiles [hd, seq]
            kTr = acts.tile([128, S], bf16)
            v_sb = acts.tile([128, S], bf16)         # 16 [seq,hd] tiles at jt*128

            # deferred per-head epilogue (bc matmul + normalize + store), run
            # one head late so the PE never waits on the 1/l ACT chain
            pending = []
            xpre = [None] * 16   # next-chunk x tiles, prefetched during attn

            def flush_epilogue():
                if not pending:
                    return
                pg, ph, p_oT, p_linv = pending.pop()
                bc_ps = pbank(7, name="bc")
                nc.tensor.matmul(bc_ps[:], onesr_sb[:], p_linv[:],
                                 start=True, stop=True)
                bc_sb = epi.tile([128, 512], bf16, tag="bcsb")
                # copy on DVE: the ACT queue is the attention pacer (exps)
                nc.vector.tensor_copy(out=bc_sb[:], in_=bc_ps[:])
                oT_sb = epi.tile([128, 512], bf16, tag="otsb", bufs=5)
                nc.vector.tensor_mul(oT_sb[:], p_oT[:], bc_sb[:])
                nc.sync.dma_start(ag_ins[pg][bass.ts(ph, 128)], oT_sb[:])
                if ph == QH - 1:
                    nc.gpsimd.collective_compute(
                        "AllGather", mybir.AluOpType.bypass,
                        replica_groups=[list(range(NCORES))],
                        ins=[ag_ins[pg][:]], outs=[ag_outs[pg][:]],
                    )

            # ---- fused phase A+B: per chunk n, projections + rope, then
            # ---- attention group g=n (needs only K/V through chunk n)
            for n in range(NSEQ):
                sl = bass.ts(n, 512)
                q_ps = [pbank(m, name="q") for m in range(QH)]
                k_ps = pbank(4, name="k")
                vT_ps = pbank(5, name="vT")
                if n == 0:
                    # fine-grained wq/x interleave on SP so tile k lands in
                    # time for the k-th accumulation step; rope tables and
                    # small constants slotted in behind the first pairs; the
                    # k/v matmuls trail the q matmuls by KVD steps so wk/wv/x
                    # arrivals are off the cold-ramp critical path
                    KVD = 4
                    xs = []

                    def kv_mms(kk):
                        stq, spq = (kk == 0), (kk == KT - 1)
                        nc.tensor.matmul(k_ps[:], wk_sb[:, kk], xs[kk][:],
                                         start=stq, stop=spq)
                        nc.tensor.matmul(vT_ps[:], wv_sb[:, kk], xs[kk][:],
                                         start=stq, stop=spq)

                    for k in range(KT):
                        if k in (0, 2, 4, 6):
                            # wk/wv in 256KB pieces, on SP: the GpSimd queue
                            # sits behind the startup barrier
                            p = k // 2
                            nc.sync.dma_start(wk_sb[:, 8 * p:8 * p + 8],
                                              wkr[:, 8 * p:8 * p + 8])
                            nc.sync.dma_start(wv_sb[:, 8 * p:8 * p + 8],
                                              wvr[:, 8 * p:8 * p + 8])
                        x_sb = xin.tile([128, 512], bf16, tag="x", bufs=8)
                        xq = (nc.sync, nc.scalar)[k % 2] if k < 12 else nc.sync
                        if k < 2:
                            # halved first tiles land ~3us sooner
                            xq.dma_start(x_sb[:, 0:256],
                                         xr[:, k, bass.ds(512 * n, 256)])
                            xq.dma_start(x_sb[:, 256:512],
                                         xr[:, k, bass.ds(512 * n + 256, 256)])
                            nc.sync.dma_start(wq_sb[:, k, 0:256],
                                              wqr[:, k, 0:256])
                            nc.sync.dma_start(wq_sb[:, k, 256:512],
                                              wqr[:, k, 256:512])
                        else:
                            xq.dma_start(x_sb[:], xr[:, k, sl])
                            nc.sync.dma_start(wq_sb[:, k], wqr[:, k])
                        xs.append(x_sb)
                        if k == 1:
                            nc.sync.dma_start(cos_sb[:, 0:1024], cosT[:, 0:1024])
                            nc.sync.dma_start(cos_sb[:, 1024:2048],
                                              cosT[:, 1024:2048])
                            nc.sync.dma_start(sin_sb[:, 0:1024], sinT[:, 0:1024])
                            nc.sync.dma_start(sin_sb[:, 1024:2048],
                                              sinT[:, 1024:2048])
                            nc.sync.dma_start(perm_sb[:], perm[:])
                        if k == 8:
                            nc.sync.dma_start(ident_sb[:], ident[:])
                            nc.sync.dma_start(tri_sb[:], tri[:])
                        st, sp = (k == 0), (k == KT - 1)
                        for m in range(QH):
                            nc.tensor.matmul(q_ps[m][:], wq_sb[:, k, bass.ts(m, 128)],
                                             x_sb[:], start=st, stop=sp)
                        if k >= KVD:
                            kv_mms(k - KVD)
                    for k in range(KT - KVD, KT):
                        kv_mms(k)
                else:
                    # batched x: 16 pieces of 2 k-tiles (256KB each), issued
                    # alternately from SP and GpSimd (idle after the startup
                    # barrier) so neither queue paces the PE
                    # x tiles were prefetch-issued during the previous
                    # attention group (SP queue); consume them here
                    for kb in range(16):
                        x2 = xpre[kb]
                        if x2 is None:
                            x2 = xin.tile([128, 2, 512], bf16, tag="x2",
                                          bufs=12)
                            nc.sync.dma_start(x2[:],
                                              xr[:, 2 * kb:2 * kb + 2, sl])
                        for j in range(2):
                            k = 2 * kb + j
                            st, sp = (k == 0), (k == KT - 1)
                            for m in range(QH):
                                nc.tensor.matmul(q_ps[m][:],
                                                 wq_sb[:, k, bass.ts(m, 128)],
                                                 x2[:, j], start=st, stop=sp)
                            if k == 0:
                                # previous group's last epilogue: the k=0 q
                                # matmuls above give the PE runway for its 1/l
                                # chain; it must flush before the k/v matmuls
                                # below reuse PSUM banks 4/5
                                flush_epilogue()
                            nc.tensor.matmul(k_ps[:], wk_sb[:, k], x2[:, j],
                                             start=st, stop=sp)
                            nc.tensor.matmul(vT_ps[:], wv_sb[:, k], x2[:, j],
                                             start=st, stop=sp)
                    if n == 1:
                        # wo (needed by phase C) behind chunk-1 x traffic
                        for p in range(8):
                            nc.sync.dma_start(wo_sb[:, 4 * p:4 * p + 4],
                                              wor[:, 4 * p:4 * p + 4])

                # prefetch the NEXT chunk's x tiles now, all on SP: emitted
                # here they issue during this attention group, so a DMA-ring
                # conflict with an in-flight AllGather can no longer starve
                # the next chunk's projections (the GpSimd queue stays empty
                # of DMAs for the same reason)
                if n + 1 < NSEQ:
                    nxt = []
                    for kb in range(16):
                        x2n = xin.tile([128, 2, 512], bf16, tag="x2", bufs=12)
                        nc.sync.dma_start(
                            x2n[:],
                            xr[:, 2 * kb:2 * kb + 2, bass.ts(n + 1, 512)])
                        nxt.append(x2n)
                    xpre = nxt

                # rope: q0 first (attention head 0 needs it first), then k
                # (needed by head 0's last j-tiles), then q1..q3; per tensor,
                # first free the accumulation bank (copy + cos-mul), then the
                # sw-product and adds
                order = [0, QH] + list(range(1, QH))   # q0, k, q1, q2, q3
                t_bfs, t1s = {}, {}
                for idx in order:
                    src = q_ps[idx] if idx < QH else k_ps
                    t_bf = rope.tile([128, 512], bf16, tag=f"tbf{idx}",
                                     name=f"tbf{idx}", bufs=1)
                    nc.scalar.copy(t_bf[:], src[:])
                    t1 = rope.tile([128, 512], bf16, tag=f"t1_{idx}",
                                   name=f"t1_{idx}", bufs=1)
                    # all-bf16 cos-mul: 2x DVE rate, and the PSUM bank is
                    # released by the ACT copy alone
                    nc.vector.tensor_mul(t1[:], t_bf[:], cos_sb[:, sl])
                    t_bfs[idx] = t_bf
                    t1s[idx] = t1
                for i, idx in enumerate(order):
                    dst = qTr[:, idx, sl] if idx < QH else kTr[:, sl]
                    sw_ps = pbank(6 + (i % 2), name="sw")
                    nc.tensor.matmul(sw_ps[:], perm_sb[:], t_bfs[idx][:],
                                     start=True, stop=True)
                    # bf16 t2 keeps the final add at the DVE's 2x 16-bit rate
                    t2 = rope.tile([128, 512], bf16, tag=f"t2_{i % 2}",
                                   name=f"t2_{i % 2}")
                    nc.vector.tensor_mul(t2[:], sw_ps[:], sin_sb[:, sl])
                    nc.vector.tensor_add(dst, t1s[idx][:], t2[:])

                # v: copy vT chunk, transpose 128-blocks into [seq, hd] tiles
                v_bf = rope.tile([128, 512], bf16, tag="vbf")
                nc.scalar.copy(v_bf[:], vT_ps[:])
                for t in range(4):
                    vt_ps = pbank(6 + (t % 2), shape=(128, 128), dtype=bf16,
                                  name="vt")
                    nc.tensor.transpose(vt_ps[:], v_bf[:, bass.ts(t, 128)],
                                        ident_sb[:])
                    nc.any.tensor_copy(out=v_sb[:, bass.ts(4 * n + t, 128)],
                                       in_=vt_ps[:])

                # ---- attention group g = n ----
                # banks: st rotation 0/1/2, oT 3/4, l 5/6, bc 7
                g = n
                njt = 4 * g + 4
                for h in range(QH):
                    oT_ps = pbank(3 if h % 2 == 0 else 4, name="oT")
                    l_ps = pbank(5 if h % 2 == 0 else 6, shape=(1, 512), name="l")
                    STB = (0, 1, 2)
                    LAG = 3   # l/oT consume pt three tiles behind st/exp so
                    #           the PE never blocks on the ACT exp latency
                    pts = []
                    # l items: full j-tile PAIRS are pre-summed on the DVE
                    # (bf16, 2x rate) so the l matmuls stream half the
                    # columns; diagonal partials feed l directly
                    lits = []       # (ap, c0) consumed by the l matmuls
                    lit_tile = []   # last j-tile covered by lits[k]
                    nlit = 2 * g + 4

                    def emit_l(ks):
                        # consecutive l matmuls share the ones stationary
                        for k in ks:
                            p_ap, p_c0 = lits[k]
                            nc.tensor.matmul(l_ps[:, p_c0:512], ones_sb[:],
                                             p_ap[:, p_c0:512],
                                             start=(k == 0),
                                             stop=(k == nlit - 1))

                    def emit_o(js):
                        for j in js:
                            p_pt, p_c0 = pts[j]
                            nc.tensor.matmul(oT_ps[:, p_c0:512],
                                             v_sb[:, bass.ts(j, 128)],
                                             p_pt[:, p_c0:512],
                                             start=(j == 0),
                                             stop=(j == njt - 1))

                    done_o = 0
                    done_l = 0
                    for jt in range(njt):
                        r = jt - 4 * g
                        c0 = max(r, 0) * 128   # first live column in i-chunk
                        isl = bass.ds(512 * g + c0, 512 - c0)
                        st_ps = pbank(STB[jt % 3], name="st")
                        nc.tensor.matmul(st_ps[:, c0:512],
                                         kTr[:, bass.ts(jt, 128)],
                                         qTr[:, h, isl], start=True, stop=True)
                        pt = ptp.tile([128, 512], bf16, tag="pt")
                        nc.scalar.activation(pt[:, c0:512], st_ps[:, c0:512],
                                             mybir.ActivationFunctionType.Exp,
                                             scale=SCALE)
                        if r >= 0:
                            # causal mask on the first live 128 cols (0/1 mul)
                            nc.vector.tensor_mul(pt[:, c0:c0 + 128],
                                                 pt[:, c0:c0 + 128], tri_sb[:])
                            lits.append((pt, c0))
                            lit_tile.append(jt)
                        elif jt % 2 == 1:
                            ls = ptp.tile([128, 512], bf16, tag="lsum",
                                          bufs=4)
                            nc.vector.tensor_add(ls[:], pts[jt - 1][0][:],
                                                 pt[:])
                            lits.append((ls, 0))
                            lit_tile.append(jt)
                        pts.append((pt, c0))
                        if jt >= LAG and (jt - LAG - done_o) >= 1:
                            hi = jt - LAG + 1
                            l_hi = sum(1 for t in lit_tile if t < hi)
                            if l_hi > done_l:
                                emit_l(range(done_l, l_hi))
                                done_l = l_hi
                            emit_o(range(done_o, hi))
                            done_o = hi
                        if jt == min(2, njt - 1):
                            # previous head's epilogue, now that the PE has
                            # runway (its 1/l ACT chain is long done)
                            flush_epilogue()
                    if done_l < nlit:
                        emit_l(range(done_l, nlit))
                    if done_o < njt:
                        emit_o(range(done_o, njt))
                    # 1/l = exp(-ln l) entirely on ACT: keeps the in-order ACT
                    # queue free of cross-engine waits; the PE-side broadcast
                    # and normalize are deferred one head
                    lnl = epi.tile([1, 512], f32, tag="lnl")
                    nc.scalar.activation(lnl[:], l_ps[:],
                                         mybir.ActivationFunctionType.Ln)
                    linv_bf = epi.tile([1, 512], bf16, tag="linvbf")
                    nc.scalar.activation(linv_bf[:], lnl[:],
                                         mybir.ActivationFunctionType.Exp,
                                         scale=-1.0)
                    pending.append((g, h, oT_ps, linv_bf))
                if n == NSEQ - 1:
                    # no next chunk to carry the deferred epilogue
                    flush_epilogue()

            # ---- phase C: outT = wo_c^T @ Y^T, wo stationary from SBUF ----
            # tile_wait_until pins every phase-C instruction to the END of its
            # engine queue: without it the list scheduler (whose cost model
            # assumes fast collectives) hoists the AG-gated y loads to the
            # HEAD of the in-order SP/ACT queues, where the real AllGather
            # latency (launch skew) head-of-line-blocks the x prefetches, ag
            # stores and rope copies queued behind them (~35us PE stall).
            for ns in range(NSEQ):
                with tc.tile_wait_until(1.0 + 0.2 * ns):
                    o_ps = [pbank((0 if ns % 2 == 0 else 4) + ob, name="o")
                            for ob in range(QH)]
                    ys = []
                    for kt in range(KT):
                        c, db = kt // 4, kt % 4
                        y_sb = cproj.tile([128, 512], bf16, tag="y")
                        # ALL y loads on SP: the ACT queue still runs
                        # attention-3 exps when chunk-0's y tiles are due,
                        # so ACT-queued loads arrive late and stall phase C
                        nc.sync.dma_start(y_sb[:],
                                          ag_outs[ns][c, bass.ts(db, 128)])
                        ys.append(y_sb)
                    if ns < NSEQ - 1:
                        for kt in range(KT):
                            for ob in range(QH):
                                nc.tensor.matmul(
                                    o_ps[ob][:], wo_sb[:, kt, bass.ts(ob, 128)],
                                    ys[kt][:],
                                    start=(kt == 0), stop=(kt == KT - 1))
                    else:
                        # last chunk ob-major: each head-block finishes its
                        # full contraction early so its copy+store pipeline
                        # under the remaining matmuls (shorter drain tail)
                        for ob in range(QH):
                            for kt in range(KT):
                                nc.tensor.matmul(
                                    o_ps[ob][:], wo_sb[:, kt, bass.ts(ob, 128)],
                                    ys[kt][:],
                                    start=(kt == 0), stop=(kt == KT - 1))
                            o_sb = cproj.tile([128, 512], bf16, tag="osb",
                                              bufs=4)
                            nc.vector.tensor_copy(out=o_sb[:], in_=o_ps[ob][:])
                            oq = nc.sync if ob % 2 == 0 else nc.scalar
                            oq.dma_start(outT[bass.ts(ob, 128),
                                              bass.ts(ns, 512)], o_sb[:])
                if ns == NSEQ - 1:
                    continue
                # stores staggered AFTER chunk ns+1's y loads (anchor 1.3+.2ns
                # vs loads at 1.0+.2ns) so they never head-of-line block the
                # y prefetch on the in-order SP/ACT queues; PSUM evacuation
                # copies go on the (idle) DVE queue for the same reason
                with tc.tile_wait_until(1.3 + 0.2 * ns):
                    for ob in range(QH):
                        o_sb = cproj.tile([128, 512], bf16, tag="osb", bufs=4)
                        nc.vector.tensor_copy(out=o_sb[:], in_=o_ps[ob][:])
                        oq = nc.sync if ob % 2 == 0 else nc.scalar
                        oq.dma_start(outT[bass.ts(ob, 128), bass.ts(ns, 512)],
                                     o_sb[:])

    _legalize_waits(nc)
    return nc


def _host_inputs(x, wq, wk, wv, wo):
    x = np.asarray(x, dtype=np.float32)
    xT = np.ascontiguousarray(x.reshape(S, DIM).T).astype(ml_dtypes.bfloat16)

    # rope tables in [hd, seq] layout with the sign of sin baked in
    inv_freq = 1.0 / ROPE_BASE ** (np.arange(0, HD, 2, dtype=np.float32) / HD)
    t = np.arange(S, dtype=np.float32)
    freqs = np.outer(inv_freq, t)                       # [64, S]
    cosT = np.concatenate([np.cos(freqs), np.cos(freqs)], 0)
    sinT = np.concatenate([-np.sin(freqs), np.sin(freqs)], 0)

    # 0/1 causal mask for a 128x128 diagonal block: keep j <= i
    j = np.arange(128)[:, None]
    i = np.arange(128)[None, :]
    tri = (j <= i).astype(np.float32)

    perm = np.zeros((128, 128), dtype=np.float32)
    perm[np.arange(128), (np.arange(128) + 64) % 128] = 1.0
    ident = np.eye(128, dtype=np.float32)

    shared = {
        "xT": xT,
        "cosT": cosT.astype(ml_dtypes.bfloat16),
        "sinT": sinT.astype(ml_dtypes.bfloat16),
        "tri": tri.astype(ml_dtypes.bfloat16),
        "perm": perm.astype(ml_dtypes.bfloat16),
        "ident": ident.astype(ml_dtypes.bfloat16),
    }
    maps = []
    for c in range(NCORES):
        m = dict(shared)
        m["wq"] = np.asarray(wq[:, c * QW:(c + 1) * QW]).astype(ml_dtypes.bfloat16)
        m["wk"] = np.asarray(wk[:, c * HD:(c + 1) * HD]).astype(ml_dtypes.bfloat16)
        m["wv"] = np.asarray(wv[:, c * HD:(c + 1) * HD]).astype(ml_dtypes.bfloat16)
        m["wo"] = np.asarray(wo[:, c * QW:(c + 1) * QW]).astype(ml_dtypes.bfloat16)
        maps.append(m)
    return maps


LAST_RESULT = {}


def kernel(x, wq, wk, wv, wo, mask=None, trace=False):
    if "nc" not in _CACHE:
        _CACHE["nc"] = _build_nc()
    nc = _CACHE["nc"]
    in_maps = _host_inputs(x, wq, wk, wv, wo)
    res = run_bass_kernel_spmd(nc, in_maps, list(range(NCORES)), trace=trace)
    LAST_RESULT["exec_time_ns"] = res.exec_time_ns
    LAST_RESULT["profile_json"] = res.profile_json
    it = res.instructions_and_trace
    LAST_RESULT["trace_dir"] = it if isinstance(it, str) else None
    full = np.concatenate(
        [res.results[c]["outT"].astype(np.float32).T for c in range(NCORES)],
        axis=1)
    return np.ascontiguousarray(full).reshape(1, S, DIM).astype(np.float32)



# revision 15
# speedup vs baseline: 1.0353x; 1.0353x over previous
"""GQA causal attention block (RoPE, 32 Q heads / 8 KV heads, S=2048, D=4096)
distributed tensor-parallel over heads across 8 TRN2 NeuronCores.

Per core c:
  - 4 query heads (wq cols 512c:512c+512), 1 KV head (wk/wv cols 128c:128c+128)
  - projections computed transposed (qT/kT/vT [hd, seq]) from host-transposed
    xT, weights as stationary operands, bf16 matmuls with f32 PSUM accum
  - RoPE applied with host tables (bf16); the half-rotation uses a PE
    permutation matmul (cross-partition moves are impossible on DVE)
  - attention group g (i-chunk g of 512) runs immediately after projection
    chunk g — causality means it only needs K/V through chunk g — so each
    chunked AllGather launches hundreds of us earlier and inter-core launch
    skew is absorbed by the remaining compute instead of stalling the PE
  - attention computed as S^T [j, i] tiles -> exp (ACT, scale folded in) ->
    P^T tiles feed the PV matmul directly; causal handled by skipping future
    j-tiles, streaming only the live column subrange on diagonal tiles, and
    one DVE 0/1 triangular mask multiply on the first 128 live columns; the
    l/oT accumulation trails st/exp by 3 tiles so the PE never blocks on the
    ACT exp latency
  - row sums via a ones-vector matmul; 1/l = exp(-ln l) computed entirely on
    ACT (keeps the in-order ACT queue free of cross-engine waits), broadcast
    across partitions with a K=1 PE matmul (no DRAM round-trip); the
    broadcast matmul is software-pipelined one head late
  - output projection computed transposed (outT = wo_c^T @ Y^T); outT is
    written bf16 and upcast on host
  - bulk DMA alternates between the SP and ACT issue queues, ordered so
    chunk-0 operands land first; the GpSimd queue carries ONLY the four
    AllGather triggers (a trigger parked on the serial CC stream under launch
    skew would otherwise head-of-line-block DMAs queued behind it)
Host gathers by concatenating the 8 (transposed) column shards.
"""

import numpy as np
import ml_dtypes

import concourse.bass as bass
import concourse.mybir as mybir
import concourse.tile as tile
from concourse.bass_utils import run_bass_kernel_spmd

bf16 = mybir.dt.bfloat16
f32 = mybir.dt.float32

NCORES = 8
S = 2048
DIM = 4096
HD = 128
NH = 32
QH = NH // NCORES          # 4 query heads per core
QW = QH * HD               # 512 wq cols per core
ROPE_BASE = 10000.0
SCALE = float(HD) ** -0.5
NSEQ = S // 512            # 4 seq chunks
KT = DIM // 128            # 32 contraction tiles

_CACHE = {}


def _legalize_waits(nc, allowed_default=1):
    """This walrus build rejects instructions carrying more inline sync waits
    than the opcode template allows (0 for Drain, 1 elsewhere). Spill excess
    waits onto standalone EventSemaphore instructions inserted immediately
    before, on the same engine (engine order preserves semantics)."""
    for f in nc.m.functions:
        for bb in f.blocks:
            out = []
            for ins in bb.instructions:
                tname = type(ins).__name__
                si = getattr(ins, "sync_info", None)
                waits = list(si.on_wait) if (si is not None and si.on_wait) else []
                if tname == "InstEventSemaphore":
                    allowed = len(waits)
                elif tname == "InstDrain":
                    allowed = 0
                else:
                    allowed = allowed_default
                if len(waits) > allowed:
                    spill, keep = waits[allowed:], waits[:allowed]
                    for i, w in enumerate(spill):
                        ev = mybir.InstEventSemaphore(
                            name=f"{ins.name}_wfix{i}",
                            engine=ins.engine, ins=[], outs=[],
                        )
                        ev.sync_info = mybir.SyncInfo(on_wait=[w], on_update=[])
                        out.append(ev)
                    si.on_wait = keep
                out.append(ins)
            bb.instructions[:] = out


def _build_nc():
    nc = bass.Bass(num_devices=NCORES)

    xT = nc.declare_dram_parameter("xT", [DIM, S], bf16, isOutput=False)
    wq = nc.declare_dram_parameter("wq", [DIM, QW], bf16, isOutput=False)
    wk = nc.declare_dram_parameter("wk", [DIM, HD], bf16, isOutput=False)
    wv = nc.declare_dram_parameter("wv", [DIM, HD], bf16, isOutput=False)
    wo = nc.declare_dram_parameter("wo", [DIM, QW], bf16, isOutput=False)
    cosT = nc.declare_dram_parameter("cosT", [HD, S], bf16, isOutput=False)
    sinT = nc.declare_dram_parameter("sinT", [HD, S], bf16, isOutput=False)
    tri = nc.declare_dram_parameter("tri", [128, 128], bf16, isOutput=False)
    perm = nc.declare_dram_parameter("perm", [128, 128], bf16, isOutput=False)
    ident = nc.declare_dram_parameter("ident", [128, 128], bf16, isOutput=False)
    outT = nc.declare_dram_parameter("outT", [QW, S], bf16, isOutput=True)

    ag_ins = [nc.dram_tensor(f"ag_in{g}", [QW, 512], bf16) for g in range(3)]
    ag_outs = [
        nc.dram_tensor(f"ag_out{g}", [NCORES, QW, 512], bf16, addr_space="Shared")
        for g in range(3)
    ]
    # the LAST group's AllGather is split: heads 0-2 ship as soon as head 2's
    # epilogue lands (overlapping head 3's attention), so only head 3's small
    # 128-row gather sits on the end-of-kernel critical path under core skew
    ag_in3a = nc.dram_tensor("ag_in3a", [3 * 128, 512], bf16)
    ag_out3a = nc.dram_tensor("ag_out3a", [NCORES, 3 * 128, 512], bf16,
                              addr_space="Shared")
    ag_in3b = nc.dram_tensor("ag_in3b", [128, 512], bf16)
    ag_out3b = nc.dram_tensor("ag_out3b", [NCORES, 128, 512], bf16,
                              addr_space="Shared")

    with tile.TileContext(nc) as tc:
        with (
            tc.tile_pool(name="const", bufs=1) as constp,
            tc.tile_pool(name="acts", bufs=1) as acts,
            tc.tile_pool(name="xin", bufs=6) as xin,
            tc.tile_pool(name="rope", bufs=2) as rope,
            tc.tile_pool(name="pt", bufs=6) as ptp,
            tc.tile_pool(name="epi", bufs=3) as epi,
            tc.tile_pool(name="cproj", bufs=12) as cproj,
            tc.tile_pool(name="psum", bufs=1, space="PSUM") as psum,
        ):
            def pbank(b, shape=(128, 512), dtype=f32, name="ps"):
                return psum.tile(list(shape), dtype, tag=f"b{b}",
                                 name=f"{name}_b{b}", bufs=1)

            # --- PE warmup: ~12 dummy matmuls on a memset tile fill the
            # 9-13us NEFF-startup window (before the first DMAs land) with
            # back-to-back PE work, flipping the HAM clock gate to K=16/16
            # (2.4GHz) BEFORE the real chunk-0 projections start; without
            # this the first ~70 matmuls run at the cold 1.2GHz clock ---
            warm_sb = constp.tile([128, 512], bf16)
            nc.vector.memset(warm_sb[:], 0.015625)
            warm_ps = pbank(7, name="warm")
            for _ in range(8):
                nc.tensor.matmul(warm_ps[:], warm_sb[:, 0:128], warm_sb[:],
                                 start=True, stop=True)

            # --- constants / weights (SBUF homes) ---
            wq_sb = constp.tile([128, KT, QW], bf16)
            wk_sb = constp.tile([128, KT, HD], bf16)
            wv_sb = constp.tile([128, KT, HD], bf16)
            wo_sb = constp.tile([128, KT, QW], bf16)
            cos_sb = constp.tile([HD, S], bf16)
            sin_sb = constp.tile([HD, S], bf16)
            tri_sb = constp.tile([128, 128], bf16)
            perm_sb = constp.tile([128, 128], bf16)
            ident_sb = constp.tile([128, 128], bf16)
            ones_sb = constp.tile([128, 1], bf16)
            onesr_sb = constp.tile([1, 128], bf16)
            nc.vector.memset(ones_sb[:], 1.0)
            nc.vector.memset(onesr_sb[:], 1.0)

            wqr = wq.rearrange("(a p) m -> p a m", p=128)
            wkr = wk.rearrange("(a p) m -> p a m", p=128)
            wvr = wv.rearrange("(a p) m -> p a m", p=128)
            wor = wo.rearrange("(a p) m -> p a m", p=128)
            xr = xT.rearrange("(a p) m -> p a m", p=128)

            # activations that live through attention
            qTr = acts.tile([128, QH, S], bf16)      # 4 head tiles [hd, seq]
            kTr = acts.tile([128, S], bf16)
            v_sb = acts.tile([128, S], bf16)         # 16 [seq,hd] tiles at jt*128

            # deferred per-head epilogue (bc matmul + normalize + store), run
            # one head late so the PE never waits on the 1/l ACT chain
            pending = []
            xpre = [None] * 16   # next-chunk x tiles, prefetched during attn

            def flush_epilogue():
                if not pending:
                    return
                pg, ph, p_oT, p_linv = pending.pop()
                bc_ps = pbank(7, name="bc")
                nc.tensor.matmul(bc_ps[:], onesr_sb[:], p_linv[:],
                                 start=True, stop=True)
                bc_sb = epi.tile([128, 512], bf16, tag="bcsb")
                # copy on DVE: the ACT queue is the attention pacer (exps)
                nc.vector.tensor_copy(out=bc_sb[:], in_=bc_ps[:])
                oT_sb = epi.tile([128, 512], bf16, tag="otsb", bufs=5)
                nc.vector.tensor_mul(oT_sb[:], p_oT[:], bc_sb[:])
                if pg < NSEQ - 1:
                    nc.sync.dma_start(ag_ins[pg][bass.ts(ph, 128)], oT_sb[:])
                    if ph == QH - 1:
                        nc.gpsimd.collective_compute(
                            "AllGather", mybir.AluOpType.bypass,
                            replica_groups=[list(range(NCORES))],
                            ins=[ag_ins[pg][:]], outs=[ag_outs[pg][:]],
                        )
                elif ph < QH - 1:
                    nc.sync.dma_start(ag_in3a[bass.ts(ph, 128)], oT_sb[:])
                    if ph == QH - 2:
                        nc.gpsimd.collective_compute(
                            "AllGather", mybir.AluOpType.bypass,
                            replica_groups=[list(range(NCORES))],
                            ins=[ag_in3a[:]], outs=[ag_out3a[:]],
                        )
                else:
                    nc.sync.dma_start(ag_in3b[:], oT_sb[:])
                    nc.gpsimd.collective_compute(
                        "AllGather", mybir.AluOpType.bypass,
                        replica_groups=[list(range(NCORES))],
                        ins=[ag_in3b[:]], outs=[ag_out3b[:]],
                    )

            # ---- fused phase A+B: per chunk n, projections + rope, then
            # ---- attention group g=n (needs only K/V through chunk n)
            for n in range(NSEQ):
                sl = bass.ts(n, 512)
                q_ps = [pbank(m, name="q") for m in range(QH)]
                k_ps = pbank(4, name="k")
                vT_ps = pbank(5, name="vT")
                if n == 0:
                    # chunk-0 is DMA-ISSUE-RATE bound, not bandwidth bound
                    # (the old per-k-tile schedule put 120 small DMAs on SP
                    # at ~600ns sequencer cost each while ACT idled). Load
                    # with few, large DMAs split evenly across SP+ACT: first
                    # two x/wq k-tiles halved for the fastest first matmul,
                    # then 1MB x pairs and 5-tile wq batches; the k/v matmuls
                    # trail the q matmuls by KVD steps
                    KVD = 4
                    xs = []

                    def kv_mms(kk):
                        stq, spq = (kk == 0), (kk == KT - 1)
                        nc.tensor.matmul(k_ps[:], wk_sb[:, kk], xs[kk],
                                         start=stq, stop=spq)
                        nc.tensor.matmul(vT_ps[:], wv_sb[:, kk], xs[kk],
                                         start=stq, stop=spq)

                    for k in (0, 1):
                        x_sb = xin.tile([128, 512], bf16, tag="x", bufs=2)
                        nc.sync.dma_start(x_sb[:, 0:256],
                                          xr[:, k, bass.ds(0, 256)])
                        nc.sync.dma_start(x_sb[:, 256:512],
                                          xr[:, k, bass.ds(256, 256)])
                        nc.scalar.dma_start(wq_sb[:, k, 0:256],
                                            wqr[:, k, 0:256])
                        nc.scalar.dma_start(wq_sb[:, k, 256:512],
                                            wqr[:, k, 256:512])
                        xs.append(x_sb[:])
                    for p in range(15):
                        k0 = 2 + 2 * p
                        x2 = xin.tile([128, 2, 512], bf16, tag="x2", bufs=12)
                        (nc.sync, nc.scalar)[p % 2].dma_start(
                            x2[:], xr[:, k0:k0 + 2, sl])
                        xs.append(x2[:, 0])
                        xs.append(x2[:, 1])
                        q2 = (nc.scalar, nc.sync)[p % 2]
                        if p in (0, 2, 4, 6, 8, 10):
                            a = 2 + 5 * (p // 2)
                            q2.dma_start(wq_sb[:, a:a + 5], wqr[:, a:a + 5])
                        if p in (1, 3, 5, 7):
                            i8 = 8 * ((p - 1) // 2)
                            q2.dma_start(wk_sb[:, i8:i8 + 8],
                                         wkr[:, i8:i8 + 8])
                            q2.dma_start(wv_sb[:, i8:i8 + 8],
                                         wvr[:, i8:i8 + 8])
                        if p == 9:
                            q2.dma_start(cos_sb[:], cosT[:])
                            q2.dma_start(sin_sb[:], sinT[:])
                        if p == 11:
                            q2.dma_start(perm_sb[:], perm[:])
                            q2.dma_start(ident_sb[:], ident[:])
                            q2.dma_start(tri_sb[:], tri[:])
                    for k in range(KT):
                        st, sp = (k == 0), (k == KT - 1)
                        for m in range(QH):
                            nc.tensor.matmul(q_ps[m][:], wq_sb[:, k, bass.ts(m, 128)],
                                             xs[k], start=st, stop=sp)
                        if k >= KVD:
                            kv_mms(k - KVD)
                    for k in range(KT - KVD, KT):
                        kv_mms(k)
                else:
                    # batched x: 16 pieces of 2 k-tiles (256KB each), issued
                    # alternately from SP and GpSimd (idle after the startup
                    # barrier) so neither queue paces the PE
                    # x tiles were prefetch-issued during the previous
                    # attention group (SP queue); consume them here
                    for kb in range(16):
                        x2 = xpre[kb]
                        if x2 is None:
                            x2 = xin.tile([128, 2, 512], bf16, tag="x2",
                                          bufs=12)
                            nc.sync.dma_start(x2[:],
                                              xr[:, 2 * kb:2 * kb + 2, sl])
                        for j in range(2):
                            k = 2 * kb + j
                            st, sp = (k == 0), (k == KT - 1)
                            for m in range(QH):
                                nc.tensor.matmul(q_ps[m][:],
                                                 wq_sb[:, k, bass.ts(m, 128)],
                                                 x2[:, j], start=st, stop=sp)
                            if k == 0:
                                # previous group's last epilogue: the k=0 q
                                # matmuls above give the PE runway for its 1/l
                                # chain; it must flush before the k/v matmuls
                                # below reuse PSUM banks 4/5
                                flush_epilogue()
                            nc.tensor.matmul(k_ps[:], wk_sb[:, k], x2[:, j],
                                             start=st, stop=sp)
                            nc.tensor.matmul(vT_ps[:], wv_sb[:, k], x2[:, j],
                                             start=st, stop=sp)
                    if n == 1:
                        # wo (needed by phase C) behind chunk-1 x traffic
                        for p in range(8):
                            nc.sync.dma_start(wo_sb[:, 4 * p:4 * p + 4],
                                              wor[:, 4 * p:4 * p + 4])

                # prefetch the NEXT chunk's x tiles now, all on SP: emitted
                # here they issue during this attention group, so a DMA-ring
                # conflict with an in-flight AllGather can no longer starve
                # the next chunk's projections (the GpSimd queue stays empty
                # of DMAs for the same reason)
                if n + 1 < NSEQ:
                    nxt = []
                    for kb in range(16):
                        x2n = xin.tile([128, 2, 512], bf16, tag="x2", bufs=12)
                        nc.sync.dma_start(
                            x2n[:],
                            xr[:, 2 * kb:2 * kb + 2, bass.ts(n + 1, 512)])
                        nxt.append(x2n)
                    xpre = nxt

                # rope: q0 first (attention head 0 needs it first), then k
                # (needed by head 0's last j-tiles), then q1..q3; per tensor,
                # first free the accumulation bank (copy + cos-mul), then the
                # sw-product and adds
                order = [0, QH] + list(range(1, QH))   # q0, k, q1, q2, q3
                t_bfs, t1s = {}, {}
                for idx in order:
                    src = q_ps[idx] if idx < QH else k_ps
                    t_bf = rope.tile([128, 512], bf16, tag=f"tbf{idx}",
                                     name=f"tbf{idx}", bufs=1)
                    nc.scalar.copy(t_bf[:], src[:])
                    t1 = rope.tile([128, 512], bf16, tag=f"t1_{idx}",
                                   name=f"t1_{idx}", bufs=1)
                    # all-bf16 cos-mul: 2x DVE rate, and the PSUM bank is
                    # released by the ACT copy alone
                    nc.vector.tensor_mul(t1[:], t_bf[:], cos_sb[:, sl])
                    t_bfs[idx] = t_bf
                    t1s[idx] = t1
                for i, idx in enumerate(order):
                    dst = qTr[:, idx, sl] if idx < QH else kTr[:, sl]
                    sw_ps = pbank(6 + (i % 2), name="sw")
                    nc.tensor.matmul(sw_ps[:], perm_sb[:], t_bfs[idx][:],
                                     start=True, stop=True)
                    # bf16 t2 keeps the final add at the DVE's 2x 16-bit rate
                    t2 = rope.tile([128, 512], bf16, tag=f"t2_{i % 2}",
                                   name=f"t2_{i % 2}")
                    nc.vector.tensor_mul(t2[:], sw_ps[:], sin_sb[:, sl])
                    nc.vector.tensor_add(dst, t1s[idx][:], t2[:])

                # v: copy vT chunk, transpose 128-blocks into [seq, hd] tiles
                v_bf = rope.tile([128, 512], bf16, tag="vbf")
                nc.scalar.copy(v_bf[:], vT_ps[:])
                for t in range(4):
                    vt_ps = pbank(6 + (t % 2), shape=(128, 128), dtype=bf16,
                                  name="vt")
                    nc.tensor.transpose(vt_ps[:], v_bf[:, bass.ts(t, 128)],
                                        ident_sb[:])
                    nc.any.tensor_copy(out=v_sb[:, bass.ts(4 * n + t, 128)],
                                       in_=vt_ps[:])

                # ---- attention group g = n ----
                # banks: st rotation 0/1/2, oT 3/4, l 5/6, bc 7
                g = n
                njt = 4 * g + 4
                for h in range(QH):
                    oT_ps = pbank(3 if h % 2 == 0 else 4, name="oT")
                    l_ps = pbank(5 if h % 2 == 0 else 6, shape=(1, 512), name="l")
                    STB = (0, 1, 2)
                    LAG = 3   # l/oT consume pt three tiles behind st/exp so
                    #           the PE never blocks on the ACT exp latency
                    pts = []
                    # l items: full j-tile PAIRS are pre-summed on the DVE
                    # (bf16, 2x rate) so the l matmuls stream half the
                    # columns; diagonal partials feed l directly
                    lits = []       # (ap, c0) consumed by the l matmuls
                    lit_tile = []   # last j-tile covered by lits[k]
                    nlit = 2 * g + 4

                    def emit_l(ks):
                        # consecutive l matmuls share the ones stationary
                        for k in ks:
                            p_ap, p_c0 = lits[k]
                            nc.tensor.matmul(l_ps[:, p_c0:512], ones_sb[:],
                                             p_ap[:, p_c0:512],
                                             start=(k == 0),
                                             stop=(k == nlit - 1))

                    def emit_o(js):
                        for j in js:
                            p_pt, p_c0 = pts[j]
                            nc.tensor.matmul(oT_ps[:, p_c0:512],
                                             v_sb[:, bass.ts(j, 128)],
                                             p_pt[:, p_c0:512],
                                             start=(j == 0),
                                             stop=(j == njt - 1))

                    done_o = 0
                    done_l = 0
                    for jt in range(njt):
                        r = jt - 4 * g
                        c0 = max(r, 0) * 128   # first live column in i-chunk
                        isl = bass.ds(512 * g + c0, 512 - c0)
                        st_ps = pbank(STB[jt % 3], name="st")
                        nc.tensor.matmul(st_ps[:, c0:512],
                                         kTr[:, bass.ts(jt, 128)],
                                         qTr[:, h, isl], start=True, stop=True)
                        pt = ptp.tile([128, 512], bf16, tag="pt")
                        nc.scalar.activation(pt[:, c0:512], st_ps[:, c0:512],
                                             mybir.ActivationFunctionType.Exp,
                                             scale=SCALE)
                        if r >= 0:
                            # causal mask on the first live 128 cols (0/1 mul)
                            nc.vector.tensor_mul(pt[:, c0:c0 + 128],
                                                 pt[:, c0:c0 + 128], tri_sb[:])
                            lits.append((pt, c0))
                            lit_tile.append(jt)
                        elif jt % 2 == 1:
                            ls = ptp.tile([128, 512], bf16, tag="lsum",
                                          bufs=4)
                            nc.vector.tensor_add(ls[:], pts[jt - 1][0][:],
                                                 pt[:])
                            lits.append((ls, 0))
                            lit_tile.append(jt)
                        pts.append((pt, c0))
                        if jt >= LAG and (jt - LAG - done_o) >= 1:
                            hi = jt - LAG + 1
                            l_hi = sum(1 for t in lit_tile if t < hi)
                            if l_hi > done_l:
                                emit_l(range(done_l, l_hi))
                                done_l = l_hi
                            emit_o(range(done_o, hi))
                            done_o = hi
                        if jt == min(2, njt - 1):
                            # previous head's epilogue, now that the PE has
                            # runway (its 1/l ACT chain is long done)
                            flush_epilogue()
                    if done_l < nlit:
                        emit_l(range(done_l, nlit))
                    if done_o < njt:
                        emit_o(range(done_o, njt))
                    # 1/l = exp(-ln l) entirely on ACT: keeps the in-order ACT
                    # queue free of cross-engine waits; the PE-side broadcast
                    # and normalize are deferred one head
                    lnl = epi.tile([1, 512], f32, tag="lnl")
                    nc.scalar.activation(lnl[:], l_ps[:],
                                         mybir.ActivationFunctionType.Ln)
                    linv_bf = epi.tile([1, 512], bf16, tag="linvbf")
                    nc.scalar.activation(linv_bf[:], lnl[:],
                                         mybir.ActivationFunctionType.Exp,
                                         scale=-1.0)
                    pending.append((g, h, oT_ps, linv_bf))
                if n == NSEQ - 1:
                    # no next chunk to carry the deferred epilogue
                    flush_epilogue()

            # ---- phase C: outT = wo_c^T @ Y^T, wo stationary from SBUF ----
            # tile_wait_until pins every phase-C instruction to the END of its
            # engine queue: without it the list scheduler (whose cost model
            # assumes fast collectives) hoists the AG-gated y loads to the
            # HEAD of the in-order SP/ACT queues, where the real AllGather
            # latency (launch skew) head-of-line-blocks the x prefetches, ag
            # stores and rope copies queued behind them (~35us PE stall).
            for ns in range(NSEQ):
                with tc.tile_wait_until(1.0 + 0.2 * ns):
                    o_ps = [pbank((0 if ns % 2 == 0 else 4) + ob, name="o")
                            for ob in range(QH)]
                    ys = [None] * KT
                    if ns < NSEQ - 1:
                        kts = list(range(KT))
                    else:
                        # head-3 blocks last: they arrive via the small late
                        # AllGather (ag_out3b) and must neither head-of-line
                        # block the SP queue nor gate the first matmuls
                        kts = ([4 * c + db for db in range(3)
                                for c in range(NCORES)] +
                               [4 * c + 3 for c in range(NCORES)])
                    for kt in kts:
                        c, db = kt // 4, kt % 4
                        y_sb = cproj.tile([128, 512], bf16, tag="y")
                        # ALL y loads on SP: the ACT queue still runs
                        # attention-3 exps when chunk-0's y tiles are due,
                        # so ACT-queued loads arrive late and stall phase C
                        if ns < NSEQ - 1:
                            src = ag_outs[ns][c, bass.ts(db, 128)]
                        elif db < 3:
                            src = ag_out3a[c, bass.ts(db, 128)]
                        else:
                            src = ag_out3b[c]
                        nc.sync.dma_start(y_sb[:], src)
                        ys[kt] = y_sb
                    for kt in kts:
                        for ob in range(QH):
                            nc.tensor.matmul(
                                o_ps[ob][:], wo_sb[:, kt, bass.ts(ob, 128)],
                                ys[kt][:],
                                start=(kt == kts[0]), stop=(kt == kts[-1]))
                # stores staggered AFTER chunk ns+1's y loads (anchor 1.3+.2ns
                # vs loads at 1.0+.2ns) so they never head-of-line block the
                # y prefetch on the in-order SP/ACT queues; PSUM evacuation
                # copies go on the (idle) DVE queue for the same reason
                with tc.tile_wait_until(1.3 + 0.2 * ns):
                    for ob in range(QH):
                        o_sb = cproj.tile([128, 512], bf16, tag="osb", bufs=4)
                        nc.vector.tensor_copy(out=o_sb[:], in_=o_ps[ob][:])
                        oq = nc.sync if ob % 2 == 0 else nc.scalar
                        oq.dma_start(outT[bass.ts(ob, 128), bass.ts(ns, 512)],
                                     o_sb[:])

    _legalize_waits(nc)
    return nc


def _host_inputs(x, wq, wk, wv, wo):
    x = np.asarray(x, dtype=np.float32)
    xT = np.ascontiguousarray(x.reshape(S, DIM).T).astype(ml_dtypes.bfloat16)

    # rope tables in [hd, seq] layout with the sign of sin baked in
    inv_freq = 1.0 / ROPE_BASE ** (np.arange(0, HD, 2, dtype=np.float32) / HD)
    t = np.arange(S, dtype=np.float32)
    freqs = np.outer(inv_freq, t)                       # [64, S]
    cosT = np.concatenate([np.cos(freqs), np.cos(freqs)], 0)
    sinT = np.concatenate([-np.sin(freqs), np.sin(freqs)], 0)

    # 0/1 causal mask for a 128x128 diagonal block: keep j <= i
    j = np.arange(128)[:, None]
    i = np.arange(128)[None, :]
    tri = (j <= i).astype(np.float32)

    perm = np.zeros((128, 128), dtype=np.float32)
    perm[np.arange(128), (np.arange(128) + 64) % 128] = 1.0
    ident = np.eye(128, dtype=np.float32)

    shared = {
        "xT": xT,
        "cosT": cosT.astype(ml_dtypes.bfloat16),
        "sinT": sinT.astype(ml_dtypes.bfloat16),
        "tri": tri.astype(ml_dtypes.bfloat16),
        "perm": perm.astype(ml_dtypes.bfloat16),
        "ident": ident.astype(ml_dtypes.bfloat16),
    }
    maps = []
    for c in range(NCORES):
        m = dict(shared)
        m["wq"] = np.asarray(wq[:, c * QW:(c + 1) * QW]).astype(ml_dtypes.bfloat16)
        m["wk"] = np.asarray(wk[:, c * HD:(c + 1) * HD]).astype(ml_dtypes.bfloat16)
        m["wv"] = np.asarray(wv[:, c * HD:(c + 1) * HD]).astype(ml_dtypes.bfloat16)
        m["wo"] = np.asarray(wo[:, c * QW:(c + 1) * QW]).astype(ml_dtypes.bfloat16)
        maps.append(m)
    return maps


LAST_RESULT = {}


def kernel(x, wq, wk, wv, wo, mask=None, trace=False):
    if "nc" not in _CACHE:
        _CACHE["nc"] = _build_nc()
    nc = _CACHE["nc"]
    in_maps = _host_inputs(x, wq, wk, wv, wo)
    res = run_bass_kernel_spmd(nc, in_maps, list(range(NCORES)), trace=trace)
    LAST_RESULT["exec_time_ns"] = res.exec_time_ns
    LAST_RESULT["profile_json"] = res.profile_json
    it = res.instructions_and_trace
    LAST_RESULT["trace_dir"] = it if isinstance(it, str) else None
    full = np.concatenate(
        [res.results[c]["outT"].astype(np.float32).T for c in range(NCORES)],
        axis=1)
    return np.ascontiguousarray(full).reshape(1, S, DIM).astype(np.float32)



# revision 20
# speedup vs baseline: 1.0417x; 1.0062x over previous
"""GQA causal attention block (RoPE, 32 Q heads / 8 KV heads, S=2048, D=4096)
distributed tensor-parallel over heads across 8 TRN2 NeuronCores.

Per core c:
  - 4 query heads (wq cols 512c:512c+512), 1 KV head (wk/wv cols 128c:128c+128)
  - projections computed transposed (qT/kT/vT [hd, seq]) from host-transposed
    xT, weights as stationary operands, bf16 matmuls with f32 PSUM accum
  - RoPE applied with host tables (bf16); the half-rotation uses a PE
    permutation matmul (cross-partition moves are impossible on DVE)
  - attention group g (i-chunk g of 512) runs immediately after projection
    chunk g — causality means it only needs K/V through chunk g — so each
    chunked AllGather launches hundreds of us earlier and inter-core launch
    skew is absorbed by the remaining compute instead of stalling the PE
  - attention computed as S^T [j, i] tiles -> exp (ACT, scale folded in) ->
    P^T tiles feed the PV matmul directly; causal handled by skipping future
    j-tiles, streaming only the live column subrange on diagonal tiles, and
    one DVE 0/1 triangular mask multiply on the first 128 live columns; the
    l/oT accumulation trails st/exp by 3 tiles so the PE never blocks on the
    ACT exp latency
  - row sums via a ones-vector matmul; 1/l = exp(-ln l) computed entirely on
    ACT (keeps the in-order ACT queue free of cross-engine waits), broadcast
    across partitions with a K=1 PE matmul (no DRAM round-trip); the
    broadcast matmul is software-pipelined one head late
  - output projection computed transposed (outT = wo_c^T @ Y^T); outT is
    written bf16 and upcast on host
  - bulk DMA alternates between the SP and ACT issue queues, ordered so
    chunk-0 operands land first; the GpSimd queue carries ONLY the four
    AllGather triggers (a trigger parked on the serial CC stream under launch
    skew would otherwise head-of-line-block DMAs queued behind it)
Host gathers by concatenating the 8 (transposed) column shards.
"""

import numpy as np
import ml_dtypes

import concourse.bass as bass
import concourse.mybir as mybir
import concourse.tile as tile
from concourse.bass_utils import run_bass_kernel_spmd

bf16 = mybir.dt.bfloat16
f32 = mybir.dt.float32

NCORES = 8
S = 2048
DIM = 4096
HD = 128
NH = 32
QH = NH // NCORES          # 4 query heads per core
QW = QH * HD               # 512 wq cols per core
ROPE_BASE = 10000.0
SCALE = float(HD) ** -0.5
NSEQ = S // 512            # 4 seq chunks
KT = DIM // 128            # 32 contraction tiles

_CACHE = {}


def _legalize_waits(nc, allowed_default=1):
    """This walrus build rejects instructions carrying more inline sync waits
    than the opcode template allows (0 for Drain, 1 elsewhere). Spill excess
    waits onto standalone EventSemaphore instructions inserted immediately
    before, on the same engine (engine order preserves semantics)."""
    for f in nc.m.functions:
        for bb in f.blocks:
            out = []
            for ins in bb.instructions:
                tname = type(ins).__name__
                si = getattr(ins, "sync_info", None)
                waits = list(si.on_wait) if (si is not None and si.on_wait) else []
                if tname == "InstEventSemaphore":
                    allowed = len(waits)
                elif tname == "InstDrain":
                    allowed = 0
                else:
                    allowed = allowed_default
                if len(waits) > allowed:
                    spill, keep = waits[allowed:], waits[:allowed]
                    for i, w in enumerate(spill):
                        ev = mybir.InstEventSemaphore(
                            name=f"{ins.name}_wfix{i}",
                            engine=ins.engine, ins=[], outs=[],
                        )
                        ev.sync_info = mybir.SyncInfo(on_wait=[w], on_update=[])
                        out.append(ev)
                    si.on_wait = keep
                out.append(ins)
            bb.instructions[:] = out


def _build_nc():
    nc = bass.Bass(num_devices=NCORES)

    xT = nc.declare_dram_parameter("xT", [DIM, S], bf16, isOutput=False)
    wq = nc.declare_dram_parameter("wq", [DIM, QW], bf16, isOutput=False)
    wk = nc.declare_dram_parameter("wk", [DIM, HD], bf16, isOutput=False)
    wv = nc.declare_dram_parameter("wv", [DIM, HD], bf16, isOutput=False)
    wo = nc.declare_dram_parameter("wo", [DIM, QW], bf16, isOutput=False)
    cosT = nc.declare_dram_parameter("cosT", [HD, S], bf16, isOutput=False)
    sinT = nc.declare_dram_parameter("sinT", [HD, S], bf16, isOutput=False)
    tri = nc.declare_dram_parameter("tri", [128, 128], bf16, isOutput=False)
    perm = nc.declare_dram_parameter("perm", [128, 128], bf16, isOutput=False)
    ident = nc.declare_dram_parameter("ident", [128, 128], bf16, isOutput=False)
    outT = nc.declare_dram_parameter("outT", [QW, S], bf16, isOutput=True)

    ag_ins = [nc.dram_tensor(f"ag_in{g}", [QW, 512], bf16) for g in range(3)]
    ag_outs = [
        nc.dram_tensor(f"ag_out{g}", [NCORES, QW, 512], bf16, addr_space="Shared")
        for g in range(3)
    ]
    # the LAST group's AllGather is split: heads 0-2 ship as soon as head 2's
    # epilogue lands (overlapping head 3's attention), so only head 3's small
    # 128-row gather sits on the end-of-kernel critical path under core skew
    ag_in3a = nc.dram_tensor("ag_in3a", [3 * 128, 512], bf16)
    ag_out3a = nc.dram_tensor("ag_out3a", [NCORES, 3 * 128, 512], bf16,
                              addr_space="Shared")
    ag_in3b = nc.dram_tensor("ag_in3b", [128, 512], bf16)
    ag_out3b = nc.dram_tensor("ag_out3b", [NCORES, 128, 512], bf16,
                              addr_space="Shared")

    with tile.TileContext(nc) as tc:
        with (
            tc.tile_pool(name="const", bufs=1) as constp,
            tc.tile_pool(name="acts", bufs=1) as acts,
            tc.tile_pool(name="xin", bufs=6) as xin,
            tc.tile_pool(name="rope", bufs=2) as rope,
            tc.tile_pool(name="pt", bufs=6) as ptp,
            tc.tile_pool(name="epi", bufs=3) as epi,
            tc.tile_pool(name="cproj", bufs=12) as cproj,
            tc.tile_pool(name="psum", bufs=1, space="PSUM") as psum,
        ):
            def pbank(b, shape=(128, 512), dtype=f32, name="ps"):
                return psum.tile(list(shape), dtype, tag=f"b{b}",
                                 name=f"{name}_b{b}", bufs=1)

            # --- PE warmup: ~12 dummy matmuls on a memset tile fill the
            # 9-13us NEFF-startup window (before the first DMAs land) with
            # back-to-back PE work, flipping the HAM clock gate to K=16/16
            # (2.4GHz) BEFORE the real chunk-0 projections start; without
            # this the first ~70 matmuls run at the cold 1.2GHz clock ---
            warm_sb = constp.tile([128, 512], bf16)
            nc.vector.memset(warm_sb[:], 0.015625)
            warm_ps = pbank(7, name="warm")
            for _ in range(8):
                nc.tensor.matmul(warm_ps[:], warm_sb[:, 0:128], warm_sb[:],
                                 start=True, stop=True)

            # --- constants / weights (SBUF homes) ---
            wq_sb = constp.tile([128, KT, QW], bf16)
            wk_sb = constp.tile([128, KT, HD], bf16)
            wv_sb = constp.tile([128, KT, HD], bf16)
            wo_sb = constp.tile([128, KT, QW], bf16)
            cos_sb = constp.tile([HD, S], bf16)
            sin_sb = constp.tile([HD, S], bf16)
            tri_sb = constp.tile([128, 128], bf16)
            perm_sb = constp.tile([128, 128], bf16)
            ident_sb = constp.tile([128, 128], bf16)
            ones_sb = constp.tile([128, 1], bf16)
            onesr_sb = constp.tile([1, 128], bf16)
            nc.vector.memset(ones_sb[:], 1.0)
            nc.vector.memset(onesr_sb[:], 1.0)

            wqr = wq.rearrange("(a p) m -> p a m", p=128)
            wkr = wk.rearrange("(a p) m -> p a m", p=128)
            wvr = wv.rearrange("(a p) m -> p a m", p=128)
            wor = wo.rearrange("(a p) m -> p a m", p=128)
            xr = xT.rearrange("(a p) m -> p a m", p=128)

            # activations that live through attention
            qTr = acts.tile([128, QH, S], bf16)      # 4 head tiles [hd, seq]
            kTr = acts.tile([128, S], bf16)
            v_sb = acts.tile([128, S], bf16)         # 16 [seq,hd] tiles at jt*128

            # deferred per-head epilogue (bc matmul + normalize + store), run
            # one head late so the PE never waits on the 1/l ACT chain
            pending = []
            xpre = [None] * 16   # next-chunk x tiles, prefetched during attn

            def flush_epilogue():
                if not pending:
                    return
                pg, ph, p_oT, p_linv = pending.pop()
                bc_ps = pbank(7, name="bc")
                nc.tensor.matmul(bc_ps[:], onesr_sb[:], p_linv[:],
                                 start=True, stop=True)
                bc_sb = epi.tile([128, 512], bf16, tag="bcsb")
                # copy on DVE: the ACT queue is the attention pacer (exps)
                nc.vector.tensor_copy(out=bc_sb[:], in_=bc_ps[:])
                oT_sb = epi.tile([128, 512], bf16, tag="otsb", bufs=5)
                nc.vector.tensor_mul(oT_sb[:], p_oT[:], bc_sb[:])
                if pg < NSEQ - 1:
                    nc.sync.dma_start(ag_ins[pg][bass.ts(ph, 128)], oT_sb[:])
                    if ph == QH - 1:
                        nc.gpsimd.collective_compute(
                            "AllGather", mybir.AluOpType.bypass,
                            replica_groups=[list(range(NCORES))],
                            ins=[ag_ins[pg][:]], outs=[ag_outs[pg][:]],
                        )
                elif ph < QH - 1:
                    nc.sync.dma_start(ag_in3a[bass.ts(ph, 128)], oT_sb[:])
                    if ph == QH - 2:
                        nc.gpsimd.collective_compute(
                            "AllGather", mybir.AluOpType.bypass,
                            replica_groups=[list(range(NCORES))],
                            ins=[ag_in3a[:]], outs=[ag_out3a[:]],
                        )
                else:
                    nc.sync.dma_start(ag_in3b[:], oT_sb[:])
                    nc.gpsimd.collective_compute(
                        "AllGather", mybir.AluOpType.bypass,
                        replica_groups=[list(range(NCORES))],
                        ins=[ag_in3b[:]], outs=[ag_out3b[:]],
                    )

            # ---- fused phase A+B: per chunk n, projections + rope, then
            # ---- attention group g=n (needs only K/V through chunk n)
            for n in range(NSEQ):
                sl = bass.ts(n, 512)
                q_ps = [pbank(m, name="q") for m in range(QH)]
                k_ps = pbank(4, name="k")
                vT_ps = pbank(5, name="vT")
                if n == 0:
                    # chunk-0 is DMA-ISSUE-RATE bound, not bandwidth bound
                    # (the old per-k-tile schedule put 120 small DMAs on SP
                    # at ~600ns sequencer cost each while ACT idled). Load
                    # with few, large DMAs split evenly across SP+ACT: first
                    # two x/wq k-tiles halved for the fastest first matmul,
                    # then 1MB x pairs and 5-tile wq batches; the k/v matmuls
                    # trail the q matmuls by KVD steps
                    KVD = 4
                    xs = []

                    def kv_mms(kk):
                        stq, spq = (kk == 0), (kk == KT - 1)
                        nc.tensor.matmul(k_ps[:], wk_sb[:, kk], xs[kk],
                                         start=stq, stop=spq)
                        nc.tensor.matmul(vT_ps[:], wv_sb[:, kk], xs[kk],
                                         start=stq, stop=spq)

                    # one DMA transfers at only ~22GB/s (single engine):
                    # aggregate bandwidth needs MANY concurrent DMAs, split
                    # evenly across the SP and ACT sequencers (one queue
                    # alone saturates at ~600ns/issue). Piece sizes grow
                    # with k: early tiles need low latency, late tiles
                    # need few issues.
                    qrr = [nc.sync, nc.scalar]
                    qn = [0]

                    def dq(dst, src):
                        qrr[qn[0] % 2].dma_start(dst, src)
                        qn[0] += 1

                    for k in range(KT):
                        if k < 4:        # quarters: ~6us single-DMA latency
                            x_sb = xin.tile([128, 512], bf16, tag="x", bufs=4)
                            for q4 in range(4):
                                x_sb_q = x_sb[:, 128 * q4:128 * q4 + 128]
                                dq(x_sb_q, xr[:, k, bass.ds(128 * q4, 128)])
                            xs.append(x_sb[:])
                        elif k < 12:     # halves
                            x_sb = xin.tile([128, 512], bf16, tag="xh",
                                            bufs=6)
                            dq(x_sb[:, 0:256], xr[:, k, bass.ds(0, 256)])
                            dq(x_sb[:, 256:512], xr[:, k, bass.ds(256, 256)])
                            xs.append(x_sb[:])
                        elif k % 2 == 0:  # whole pairs
                            x2 = xin.tile([128, 2, 512], bf16, tag="x2",
                                          bufs=12)
                            dq(x2[:], xr[:, k:k + 2, sl])
                            xs.append(x2[:, 0])
                            xs.append(x2[:, 1])
                        if k < 2:
                            dq(wq_sb[:, k, 0:256], wqr[:, k, 0:256])
                            dq(wq_sb[:, k, 256:512], wqr[:, k, 256:512])
                        else:
                            dq(wq_sb[:, k], wqr[:, k])
                        if k % 4 == 0:   # wk/wv in 4-tile 128KB pieces
                            dq(wk_sb[:, k:k + 4], wkr[:, k:k + 4])
                            dq(wv_sb[:, k:k + 4], wvr[:, k:k + 4])
                        if k in (13, 15):
                            h = 1024 * ((k - 13) // 2)
                            dq(cos_sb[:, h:h + 1024], cosT[:, h:h + 1024])
                            dq(sin_sb[:, h:h + 1024], sinT[:, h:h + 1024])
                        if k == 17:
                            dq(perm_sb[:], perm[:])
                        if k == 19:
                            dq(ident_sb[:], ident[:])
                            dq(tri_sb[:], tri[:])
                    for k in range(KT):
                        st, sp = (k == 0), (k == KT - 1)
                        for m in range(QH):
                            nc.tensor.matmul(q_ps[m][:], wq_sb[:, k, bass.ts(m, 128)],
                                             xs[k], start=st, stop=sp)
                        if k >= KVD:
                            kv_mms(k - KVD)
                    for k in range(KT - KVD, KT):
                        kv_mms(k)
                else:
                    # batched x: 16 pieces of 2 k-tiles (256KB each), issued
                    # alternately from SP and GpSimd (idle after the startup
                    # barrier) so neither queue paces the PE
                    # x tiles were prefetch-issued during the previous
                    # attention group (SP queue); consume them here
                    for kb in range(16):
                        x2 = xpre[kb]
                        if x2 is None:
                            x2 = xin.tile([128, 2, 512], bf16, tag="x2",
                                          bufs=12)
                            nc.sync.dma_start(x2[:],
                                              xr[:, 2 * kb:2 * kb + 2, sl])
                        for j in range(2):
                            k = 2 * kb + j
                            st, sp = (k == 0), (k == KT - 1)
                            for m in range(QH):
                                nc.tensor.matmul(q_ps[m][:],
                                                 wq_sb[:, k, bass.ts(m, 128)],
                                                 x2[:, j], start=st, stop=sp)
                            if k == 0:
                                # previous group's last epilogue: the k=0 q
                                # matmuls above give the PE runway for its 1/l
                                # chain; it must flush before the k/v matmuls
                                # below reuse PSUM banks 4/5
                                flush_epilogue()
                            nc.tensor.matmul(k_ps[:], wk_sb[:, k], x2[:, j],
                                             start=st, stop=sp)
                            nc.tensor.matmul(vT_ps[:], wv_sb[:, k], x2[:, j],
                                             start=st, stop=sp)
                    if n == 1:
                        # wo (needed by phase C) behind chunk-1 x traffic
                        for p in range(8):
                            nc.sync.dma_start(wo_sb[:, 4 * p:4 * p + 4],
                                              wor[:, 4 * p:4 * p + 4])

                # prefetch the NEXT chunk's x tiles now, all on SP: emitted
                # here they issue during this attention group, so a DMA-ring
                # conflict with an in-flight AllGather can no longer starve
                # the next chunk's projections (the GpSimd queue stays empty
                # of DMAs for the same reason)
                if n + 1 < NSEQ:
                    nxt = []
                    for kb in range(16):
                        x2n = xin.tile([128, 2, 512], bf16, tag="x2", bufs=12)
                        nc.sync.dma_start(
                            x2n[:],
                            xr[:, 2 * kb:2 * kb + 2, bass.ts(n + 1, 512)])
                        nxt.append(x2n)
                    xpre = nxt

                # rope: q0 first (attention head 0 needs it first), then k
                # (needed by head 0's last j-tiles), then q1..q3; per tensor,
                # first free the accumulation bank (copy + cos-mul), then the
                # sw-product and adds
                order = [0, QH] + list(range(1, QH))   # q0, k, q1, q2, q3
                t_bfs, t1s = {}, {}
                for idx in order:
                    src = q_ps[idx] if idx < QH else k_ps
                    t_bf = rope.tile([128, 512], bf16, tag=f"tbf{idx}",
                                     name=f"tbf{idx}", bufs=1)
                    nc.scalar.copy(t_bf[:], src[:])
                    t1 = rope.tile([128, 512], bf16, tag=f"t1_{idx}",
                                   name=f"t1_{idx}", bufs=1)
                    # all-bf16 cos-mul: 2x DVE rate, and the PSUM bank is
                    # released by the ACT copy alone
                    nc.vector.tensor_mul(t1[:], t_bf[:], cos_sb[:, sl])
                    t_bfs[idx] = t_bf
                    t1s[idx] = t1
                for i, idx in enumerate(order):
                    dst = qTr[:, idx, sl] if idx < QH else kTr[:, sl]
                    sw_ps = pbank(6 + (i % 2), name="sw")
                    nc.tensor.matmul(sw_ps[:], perm_sb[:], t_bfs[idx][:],
                                     start=True, stop=True)
                    # bf16 t2 keeps the final add at the DVE's 2x 16-bit rate
                    t2 = rope.tile([128, 512], bf16, tag=f"t2_{i % 2}",
                                   name=f"t2_{i % 2}")
                    nc.vector.tensor_mul(t2[:], sw_ps[:], sin_sb[:, sl])
                    nc.vector.tensor_add(dst, t1s[idx][:], t2[:])

                # v: copy vT chunk, transpose 128-blocks into [seq, hd] tiles
                v_bf = rope.tile([128, 512], bf16, tag="vbf")
                nc.scalar.copy(v_bf[:], vT_ps[:])
                for t in range(4):
                    vt_ps = pbank(6 + (t % 2), shape=(128, 128), dtype=bf16,
                                  name="vt")
                    nc.tensor.transpose(vt_ps[:], v_bf[:, bass.ts(t, 128)],
                                        ident_sb[:])
                    nc.any.tensor_copy(out=v_sb[:, bass.ts(4 * n + t, 128)],
                                       in_=vt_ps[:])

                # ---- attention group g = n ----
                # banks: st rotation 0/1/2, oT 3/4, l 5/6, bc 7
                g = n
                njt = 4 * g + 4
                for h in range(QH):
                    oT_ps = pbank(3 if h % 2 == 0 else 4, name="oT")
                    l_ps = pbank(5 if h % 2 == 0 else 6, shape=(1, 512), name="l")
                    STB = (0, 1, 2)
                    LAG = 3   # l/oT consume pt three tiles behind st/exp so
                    #           the PE never blocks on the ACT exp latency
                    pts = []
                    # l items: full j-tile PAIRS are pre-summed on the DVE
                    # (bf16, 2x rate) so the l matmuls stream half the
                    # columns; diagonal partials feed l directly
                    lits = []       # (ap, c0) consumed by the l matmuls
                    lit_tile = []   # last j-tile covered by lits[k]
                    nlit = 2 * g + 4

                    def emit_l(ks):
                        # consecutive l matmuls share the ones stationary
                        for k in ks:
                            p_ap, p_c0 = lits[k]
                            nc.tensor.matmul(l_ps[:, p_c0:512], ones_sb[:],
                                             p_ap[:, p_c0:512],
                                             start=(k == 0),
                                             stop=(k == nlit - 1))

                    def emit_o(js):
                        for j in js:
                            p_pt, p_c0 = pts[j]
                            nc.tensor.matmul(oT_ps[:, p_c0:512],
                                             v_sb[:, bass.ts(j, 128)],
                                             p_pt[:, p_c0:512],
                                             start=(j == 0),
                                             stop=(j == njt - 1))

                    done_o = 0
                    done_l = 0
                    for jt in range(njt):
                        r = jt - 4 * g
                        c0 = max(r, 0) * 128   # first live column in i-chunk
                        isl = bass.ds(512 * g + c0, 512 - c0)
                        st_ps = pbank(STB[jt % 3], name="st")
                        nc.tensor.matmul(st_ps[:, c0:512],
                                         kTr[:, bass.ts(jt, 128)],
                                         qTr[:, h, isl], start=True, stop=True)
                        pt = ptp.tile([128, 512], bf16, tag="pt")
                        nc.scalar.activation(pt[:, c0:512], st_ps[:, c0:512],
                                             mybir.ActivationFunctionType.Exp,
                                             scale=SCALE)
                        if r >= 0:
                            # causal mask on the first live 128 cols (0/1 mul)
                            nc.vector.tensor_mul(pt[:, c0:c0 + 128],
                                                 pt[:, c0:c0 + 128], tri_sb[:])
                            lits.append((pt, c0))
                            lit_tile.append(jt)
                        elif jt % 2 == 1:
                            ls = ptp.tile([128, 512], bf16, tag="lsum",
                                          bufs=4)
                            nc.vector.tensor_add(ls[:], pts[jt - 1][0][:],
                                                 pt[:])
                            lits.append((ls, 0))
                            lit_tile.append(jt)
                        pts.append((pt, c0))
                        if jt >= LAG and (jt - LAG - done_o) >= 1:
                            hi = jt - LAG + 1
                            l_hi = sum(1 for t in lit_tile if t < hi)
                            if l_hi > done_l:
                                emit_l(range(done_l, l_hi))
                                done_l = l_hi
                            emit_o(range(done_o, hi))
                            done_o = hi
                        if jt == min(2, njt - 1):
                            # previous head's epilogue, now that the PE has
                            # runway (its 1/l ACT chain is long done)
                            flush_epilogue()
                    if done_l < nlit:
                        emit_l(range(done_l, nlit))
                    if done_o < njt:
                        emit_o(range(done_o, njt))
                    # 1/l = exp(-ln l) entirely on ACT: keeps the in-order ACT
                    # queue free of cross-engine waits; the PE-side broadcast
                    # and normalize are deferred one head
                    lnl = epi.tile([1, 512], f32, tag="lnl")
                    nc.scalar.activation(lnl[:], l_ps[:],
                                         mybir.ActivationFunctionType.Ln)
                    linv_bf = epi.tile([1, 512], bf16, tag="linvbf")
                    nc.scalar.activation(linv_bf[:], lnl[:],
                                         mybir.ActivationFunctionType.Exp,
                                         scale=-1.0)
                    pending.append((g, h, oT_ps, linv_bf))
                if n == NSEQ - 1:
                    # no next chunk to carry the deferred epilogue
                    flush_epilogue()

            # ---- phase C: outT = wo_c^T @ Y^T, wo stationary from SBUF ----
            # tile_wait_until pins every phase-C instruction to the END of its
            # engine queue: without it the list scheduler (whose cost model
            # assumes fast collectives) hoists the AG-gated y loads to the
            # HEAD of the in-order SP/ACT queues, where the real AllGather
            # latency (launch skew) head-of-line-blocks the x prefetches, ag
            # stores and rope copies queued behind them (~35us PE stall).
            # chunk-0's first y tiles preloaded from the GpSimd queue (which
            # only carries the AG triggers): every other queue is busy with
            # attention-3 work at the moment phase C starts, so SP-queued
            # loads land ~5us after the PE is ready for them
            y0pre = []
            for kt in range(4):
                y_sb = cproj.tile([128, 512], bf16, tag="y0pre", bufs=4)
                nc.gpsimd.dma_start(y_sb[:], ag_outs[0][kt // 4, bass.ts(kt % 4, 128)])
                y0pre.append(y_sb)
            for ns in range(NSEQ):
                with tc.tile_wait_until(1.0 + 0.2 * ns):
                    o_ps = [pbank((0 if ns % 2 == 0 else 4) + ob, name="o")
                            for ob in range(QH)]
                    ys = [None] * KT
                    if ns < NSEQ - 1:
                        kts = list(range(KT))
                    else:
                        # head-3 blocks last: they arrive via the small late
                        # AllGather (ag_out3b) and must neither head-of-line
                        # block the SP queue nor gate the first matmuls
                        kts = ([4 * c + db for db in range(3)
                                for c in range(NCORES)] +
                               [4 * c + 3 for c in range(NCORES)])
                    for kt in kts:
                        if ns == 0 and kt < 4:
                            ys[kt] = y0pre[kt]
                            continue
                        c, db = kt // 4, kt % 4
                        y_sb = cproj.tile([128, 512], bf16, tag="y", bufs=8)
                        # ALL y loads on SP: the ACT queue still runs
                        # attention-3 exps when chunk-0's y tiles are due,
                        # so ACT-queued loads arrive late and stall phase C
                        if ns < NSEQ - 1:
                            src = ag_outs[ns][c, bass.ts(db, 128)]
                        elif db < 3:
                            src = ag_out3a[c, bass.ts(db, 128)]
                        else:
                            src = ag_out3b[c]
                        nc.sync.dma_start(y_sb[:], src)
                        ys[kt] = y_sb
                    for kt in kts:
                        for ob in range(QH):
                            nc.tensor.matmul(
                                o_ps[ob][:], wo_sb[:, kt, bass.ts(ob, 128)],
                                ys[kt][:],
                                start=(kt == kts[0]), stop=(kt == kts[-1]))
                # stores staggered AFTER chunk ns+1's y loads (anchor 1.3+.2ns
                # vs loads at 1.0+.2ns) so they never head-of-line block the
                # y prefetch on the in-order SP/ACT queues; PSUM evacuation
                # copies go on the (idle) DVE queue for the same reason
                with tc.tile_wait_until(1.3 + 0.2 * ns):
                    for ob in range(QH):
                        o_sb = cproj.tile([128, 512], bf16, tag="osb", bufs=4)
                        nc.vector.tensor_copy(out=o_sb[:], in_=o_ps[ob][:])
                        oq = nc.sync if ob % 2 == 0 else nc.scalar
                        oq.dma_start(outT[bass.ts(ob, 128), bass.ts(ns, 512)],
                                     o_sb[:])

    _legalize_waits(nc)
    return nc


def _host_inputs(x, wq, wk, wv, wo):
    x = np.asarray(x, dtype=np.float32)
    xT = np.ascontiguousarray(x.reshape(S, DIM).T).astype(ml_dtypes.bfloat16)

    # rope tables in [hd, seq] layout with the sign of sin baked in
    inv_freq = 1.0 / ROPE_BASE ** (np.arange(0, HD, 2, dtype=np.float32) / HD)
    t = np.arange(S, dtype=np.float32)
    freqs = np.outer(inv_freq, t)                       # [64, S]
    cosT = np.concatenate([np.cos(freqs), np.cos(freqs)], 0)
    sinT = np.concatenate([-np.sin(freqs), np.sin(freqs)], 0)

    # 0/1 causal mask for a 128x128 diagonal block: keep j <= i
    j = np.arange(128)[:, None]
    i = np.arange(128)[None, :]
    tri = (j <= i).astype(np.float32)

    perm = np.zeros((128, 128), dtype=np.float32)
    perm[np.arange(128), (np.arange(128) + 64) % 128] = 1.0
    ident = np.eye(128, dtype=np.float32)

    shared = {
        "xT": xT,
        "cosT": cosT.astype(ml_dtypes.bfloat16),
        "sinT": sinT.astype(ml_dtypes.bfloat16),
        "tri": tri.astype(ml_dtypes.bfloat16),
        "perm": perm.astype(ml_dtypes.bfloat16),
        "ident": ident.astype(ml_dtypes.bfloat16),
    }
    maps = []
    for c in range(NCORES):
        m = dict(shared)
        m["wq"] = np.asarray(wq[:, c * QW:(c + 1) * QW]).astype(ml_dtypes.bfloat16)
        m["wk"] = np.asarray(wk[:, c * HD:(c + 1) * HD]).astype(ml_dtypes.bfloat16)
        m["wv"] = np.asarray(wv[:, c * HD:(c + 1) * HD]).astype(ml_dtypes.bfloat16)
        m["wo"] = np.asarray(wo[:, c * QW:(c + 1) * QW]).astype(ml_dtypes.bfloat16)
        maps.append(m)
    return maps


LAST_RESULT = {}


def kernel(x, wq, wk, wv, wo, mask=None, trace=False):
    if "nc" not in _CACHE:
        _CACHE["nc"] = _build_nc()
    nc = _CACHE["nc"]
    in_maps = _host_inputs(x, wq, wk, wv, wo)
    res = run_bass_kernel_spmd(nc, in_maps, list(range(NCORES)), trace=trace)
    LAST_RESULT["exec_time_ns"] = res.exec_time_ns
    LAST_RESULT["profile_json"] = res.profile_json
    it = res.instructions_and_trace
    LAST_RESULT["trace_dir"] = it if isinstance(it, str) else None
    full = np.concatenate(
        [res.results[c]["outT"].astype(np.float32).T for c in range(NCORES)],
        axis=1)
    return np.ascontiguousarray(full).reshape(1, S, DIM).astype(np.float32)

